# revision 45
# baseline (speedup 1.0000x reference)
"""CrossAttention TRN2 kernel: 8-core (batch x head-group) sharded Bass/Tile implementation.

Reference computation (per batch b):
  q = x @ Wq; kv = k_in @ Wkv -> k, v   (H=16 heads, HD=64)
  attn = softmax(q k^T * HD^-0.5); attn = softmax(attn * attn_add); out = (attn @ v) @ Wproj + bproj

Sharding: core c -> batch b = c//2, heads h0 = (c%2)*8 .. +8. Each core computes a
partial (over its 8 heads) of out[b] in TRANSPOSED layout [c_out, n]; host sums the
two partials per batch, transposes back, and adds bias.

Transposed-scores pipeline (v4): scores are computed transposed (sT[m,n] = k q^T,
keys on partitions) so the attention matrix never needs a PE transpose before
attn@v. Softmax row-sums over m (the partition axis) come from PE matmuls against
an all-ones [128,128] weight, which broadcasts r1[n] across all partitions for
free (matmul cost depends only on output columns). softmax-2's normalization is
deferred through attn@v by appending a ones column to v: output column 64 of the
natural-layout [n, hd+1] product is r2[n], per-partition, so the PSUM evacuation
is a single tensor_scalar multiply by 1/r2. attn@v accumulates with M=128 (full
partition output) at half the PE cost of the [hd, n] formulation.
"""
import sys

sys.path.insert(0, "/opt/trn_rl_repo")

import numpy as np
import ml_dtypes

import concourse.bass as bass
import concourse.tile as tile
from concourse import bacc
import concourse.mybir as mybir
from concourse.bass_utils import run_bass_kernel_spmd
from concourse.masks import make_identity

B, N, C, H = 4, 1024, 1024, 16
HD = C // H          # 64
SCALE = HD ** -0.5   # 0.125
HPC = H // 2         # 8 heads per core
NT = N // 128        # 8 n-tiles
CT = C // 128        # 8 c-tiles
NP = HPC // 2        # 4 head pairs per core
BF = mybir.dt.bfloat16
F32 = mybir.dt.float32
ALU = mybir.AluOpType
AF = mybir.ActivationFunctionType

_CACHE = {}


def _build():
    nc = bacc.Bacc("TRN2", target_bir_lowering=False, debug=False, num_devices=8)
    xT = nc.declare_dram_parameter("xT", [C, N], BF, isOutput=False)
    kT = nc.declare_dram_parameter("kT", [C, N], BF, isOutput=False)
    AT = nc.declare_dram_parameter("AT", [N, N], BF, isOutput=False)
    wq = nc.declare_dram_parameter("wq", [C, HPC * HD], BF, isOutput=False)
    wk = nc.declare_dram_parameter("wk", [C, HPC * HD], BF, isOutput=False)
    wv = nc.declare_dram_parameter("wv", [C, HPC * HD], BF, isOutput=False)
    wp = nc.declare_dram_parameter("wp", [HPC * HD, C], BF, isOutput=False)
    outT = nc.declare_dram_parameter("outT", [C, N], BF, isOutput=True)

    with tile.TileContext(nc) as tc:
        _emit(nc, tc, xT, kT, AT, wq, wk, wv, wp, outT)
    nc.compile()
    return nc


def _emit(nc, tc, xT, kT, AT, wq, wk, wv, wp, outT):
    from contextlib import ExitStack

    ctx = ExitStack()
    with ctx:
        persist = ctx.enter_context(tc.tile_pool(name="persist", bufs=1))
        ph_in = ctx.enter_context(tc.tile_pool(name="ph_in", bufs=1))
        ps_s = ctx.enter_context(tc.tile_pool(name="ps_s", bufs=2, space="PSUM"))
        ps_sm = ctx.enter_context(tc.tile_pool(name="ps_sm", bufs=4, space="PSUM"))
        e1_pool = ctx.enter_context(tc.tile_pool(name="e1p", bufs=3))
        t_pool = ctx.enter_context(tc.tile_pool(name="tp", bufs=17))
        rc1_pool = ctx.enter_context(tc.tile_pool(name="rc1", bufs=2))
        rc2_pool = ctx.enter_context(tc.tile_pool(name="rc2", bufs=8))
        osb_pool = ctx.enter_context(tc.tile_pool(name="osb", bufs=8))
        oth_pool = ctx.enter_context(tc.tile_pool(name="oth", bufs=4))
        fin_pool = ctx.enter_context(tc.tile_pool(name="fin", bufs=2))

        ident = persist.tile([128, 128], BF)
        make_identity(nc, ident)
        ones = persist.tile([128, 128], BF)
        nc.gpsimd.memset(ones, 1.0)

        a_sb = persist.tile([128, NT, N], BF)      # A^T tiles: [m-chunk, n]
        qTh = persist.tile([128, NP, N], BF)       # pack p: head 2p on parts 0-63
        kTh = persist.tile([128, NP, N], BF)
        v_sb = persist.tile([128, NT, HPC, HD + 1], BF)  # per head 65 cols, col 64 = 1
        wp_sb = persist.tile([128, NP, C], BF)

        # ---- input DMAs: 4 trigger queues, first-needed first ----
        kt = ph_in.tile([128, CT, N], BF)
        wk_sb = ph_in.tile([128, CT, HPC * HD], BF)
        wv_sb = ph_in.tile([128, CT, HPC * HD], BF)
        xt = ph_in.tile([128, CT, N], BF)
        wq_sb = ph_in.tile([128, CT, HPC * HD], BF)
        kT_r = kT.rearrange("(t p) n -> p t n", p=128)
        xT_r = xT.rearrange("(t p) n -> p t n", p=128)
        AT_r = AT.rearrange("(t p) m -> p t m", p=128)
        nc.sync.dma_start(out=wk_sb, in_=wk.rearrange("(t p) m -> p t m", p=128))
        nc.scalar.dma_start(out=wq_sb, in_=wq.rearrange("(t p) m -> p t m", p=128))
        nc.gpsimd.dma_start(out=wv_sb, in_=wv.rearrange("(t p) m -> p t m", p=128))
        # Spread x/k across all three queues, low ct first (the projection
        # chains accumulate in ct order and can stream behind the DMAs).
        nc.sync.dma_start(out=kt[:, 0:3], in_=kT_r[:, 0:3])
        nc.scalar.dma_start(out=xt[:, 0:3], in_=xT_r[:, 0:3])
        nc.gpsimd.dma_start(out=kt[:, 6:8], in_=kT_r[:, 6:8])
        nc.sync.dma_start(out=xt[:, 3:6], in_=xT_r[:, 3:6])
        nc.scalar.dma_start(out=kt[:, 3:6], in_=kT_r[:, 3:6])
        nc.gpsimd.dma_start(out=xt[:, 6:8], in_=xT_r[:, 6:8])
        # A^T and Wproj ride behind the critical tensors on each queue so
        # they don't compete for bandwidth during the prologue.
        nc.sync.dma_start(out=a_sb[:, 0:4], in_=AT_r[:, 0:4])
        nc.scalar.dma_start(out=a_sb[:, 4:8], in_=AT_r[:, 4:8])
        nc.gpsimd.dma_start(out=wp_sb, in_=wp.rearrange("(t p) m -> p t m", p=128))

        nc.gpsimd.memset(v_sb[:, :, :, HD:HD + 1], 1.0)

        def act_copy(out, in_):
            nc.scalar.activation(out, in_, AF.Copy)

        def qk_chain(p, which, on_act=False):
            """One projection chain: which = 2*is_k + half."""
            is_k, half = which // 2, which % 2
            cols = bass.ts(half, 512)
            w, src_t, dst = ((wk_sb, kt, kTh) if is_k else (wq_sb, xt, qTh))
            ps = ps_s.tile([128, 512], F32, tag="s", name=f"qk{p}_{which}")
            for ct in range(CT):
                nc.tensor.matmul(
                    ps, w[:, ct, bass.ts(p, 128)], src_t[:, ct, cols],
                    start=(ct == 0), stop=(ct == CT - 1))
            (act_copy if on_act else nc.vector.tensor_copy)(dst[:, p, cols], ps)

        def qk_proj(p, on_act):
            for which in range(4):
                qk_chain(p, which, on_act)

        def v_proj_all():
            for mt in range(NT):
                v_chain(mt)

        def v_chain(mt):
            ps = ps_s.tile([128, HPC, HD], F32, tag="s", name=f"v{mt}")
            for ct in range(CT):
                nc.tensor.matmul(
                    ps, kt[:, ct, bass.ts(mt, 128)], wv_sb[:, ct, :],
                    start=(ct == 0), stop=(ct == CT - 1))
            nc.vector.tensor_copy(v_sb[:, mt, :, 0:HD], ps)

        t_tiles = {}
        e2_tiles = {}
        oth_tiles = {}
        fillers = {}

        e1_tiles = {}
        r1_tiles = {}

        def ad_block(P, mt):
            """One mt block: transposed scores -> exp1 -> t = e1 * A^T on
            DVE/Pool, plus the r1 accumulation matmuls (all-ones weight,
            broadcast over partitions) for the PREVIOUS mt block, so the
            first block of a pair can be pre-emitted into the prior pair's
            tail without holding extra r1 psum slots."""
            e1p = e1_pool.tile([128, 2, N], BF, tag="e1", name=f"e1_{P}_{mt}")
            e1_tiles[(P, mt)] = e1p
            tp = t_pool.tile([128, 2, N], BF, tag="t", name=f"t_{P}_{mt}")
            t_tiles[(P, mt)] = tp
            for hh in range(2):
                off = hh * 64
                s = ps_s.tile([128, N], F32, tag="s", name=f"s{P}_{mt}_{hh}")
                for mc in range(2):
                    nc.tensor.matmul(
                        s[:, bass.ts(mc, 512)],
                        kTh[off:off + 64, P, bass.ts(mt, 128)],
                        qTh[off:off + 64, P, bass.ts(mc, 512)],
                        start=True, stop=True)
                nc.scalar.activation(e1p[:, hh, :], s, AF.Exp, scale=SCALE)
            if mt > 0:
                ad_r1(P, mt - 1)
            for hh in range(2):
                eng = nc.vector if hh == 0 else nc.gpsimd
                eng.tensor_mul(tp[:, hh, :], e1p[:, hh, :], a_sb[:, mt, :])
            for fill in fillers.pop((P, mt), []):
                fill()

        def ad_r1(P, mt):
            e1p = e1_tiles.pop((P, mt))
            for hh in range(2):
                for half in range(2):
                    if mt == 0:
                        r1_tiles[(P, hh, half)] = ps_sm.tile(
                            [128, 512], F32, tag="sm", name=f"r1_{P}_{hh}_{half}")
                    nc.tensor.matmul(
                        r1_tiles[(P, hh, half)], ones,
                        e1p[:, hh, bass.ts(half, 512)],
                        start=(mt == 0), stop=(mt == NT - 1),
                        skip_group_check=True)

        def ad_tail(P):
            """Close the r1 chain, take reciprocals, apply *rc1 in place."""
            ad_r1(P, NT - 1)
            rc1 = rc1_pool.tile([128, 2, N], BF, tag="rc1", name=f"rc1_{P}")
            for hh in range(2):
                for half in range(2):
                    with nc.allow_low_precision(reason="softmax1 normalizer bf16"):
                        nc.vector.reciprocal(
                            rc1[:, hh, bass.ts(half, 512)],
                            r1_tiles.pop((P, hh, half)))
            for mt in range(NT):
                tp = t_tiles[(P, mt)]
                nc.vector.tensor_mul(tp[:, :, :], tp[:, :, :], rc1)

        def ad_exp2(P):
            """exp2, in place on the merged head-pair t tiles [128, 2048]."""
            for mt in range(NT):
                tp = t_tiles.pop((P, mt))
                e2_tiles[(P, mt)] = tp
                nc.scalar.activation(tp[:, :, :], tp[:, :, :], AF.Exp)

        def stage_ef(P):
            """attn@[v|1] (natural layout, col 64 = r2), normalize, transpose.
            osb packs both heads' [n,64] blocks side by side so one [128,128]
            transpose per nt yields the [2*64 hd, n] layout proj needs. PSUM
            tiles are packed two-up to halve ps_sm ring pressure."""
            oth = oth_pool.tile([128, N], BF, tag="oth", name=f"oth{P}")
            oth_tiles[P] = oth
            osbs = []

            def ef_tp(P, nt):
                pt = ps_sm.tile([128, 128], BF, tag="sm", name=f"pt{P}_{nt}")
                nc.tensor.transpose(pt, osbs[nt][:, :, :], ident)
                nc.vector.tensor_copy(oth[:, bass.ts(nt, 128)], pt)

            for nt in range(NT):
                osb = osb_pool.tile([128, 2, HD], BF, tag="osb", name=f"osb{P}_{nt}")
                osbs.append(osb)
                for hh in range(2):
                    h = 2 * P + hh
                    onat = ps_sm.tile([128, HD + 1], F32, tag="sm", name=f"o{h}_{nt}")
                    for mt in range(NT):
                        nc.tensor.matmul(
                            onat,
                            e2_tiles[(P, mt)][:, hh, bass.ts(nt, 128)],
                            v_sb[:, mt, h, :],
                            start=(mt == 0), stop=(mt == NT - 1))
                    rc2 = rc2_pool.tile([128, 1], F32, tag="rc2", name=f"rc2_{h}_{nt}")
                    nc.vector.reciprocal(rc2, onat[:, HD:HD + 1])
                    nc.vector.tensor_scalar_mul(osb[:, hh, :], onat[:, 0:HD], rc2)
                if nt >= 2:
                    ef_tp(P, nt - 2)
            ef_tp(P, NT - 2)
            ef_tp(P, NT - 1)
            for mt in range(NT):
                e2_tiles.pop((P, mt))

        # Pipeline: each pair's first block is pre-emitted into the previous
        # pair's tail so ACT's exp1 stream never stalls at the rc1/TT2
        # handoff. qk/v projection chains are spread as per-block PE fillers
        # so no single PE wall starves ACT; the exp2 tail window absorbs the
        # previous pair's output stage.
        fillers.update({
            (0, 2): [lambda: qk_chain(1, 0), lambda: qk_chain(1, 1)],
            (0, 3): [lambda: qk_chain(1, 2), lambda: qk_chain(1, 3)],
            (0, 4): [lambda: v_chain(0)], (0, 5): [lambda: v_chain(1)],
            (0, 6): [lambda: v_chain(2)], (0, 7): [lambda: v_chain(3)],
        })
        qk_proj(0, on_act=True)
        for mt in range(NT):
            ad_block(0, mt)
        ad_block(1, 0)
        ad_tail(0)
        for mt in range(4, NT):
            v_chain(mt)
        ad_exp2(0)
        for P in range(1, NP):
            if P + 1 < NP:
                fillers.update({
                    (P, 2): [lambda w=w, p=P + 1: qk_chain(p, w) for w in (0, 1)],
                    (P, 4): [lambda w=w, p=P + 1: qk_chain(p, w) for w in (2, 3)],
                })
            for mt in range(1, NT):
                ad_block(P, mt)
            if P + 1 < NP:
                ad_block(P + 1, 0)
            ad_tail(P)
            stage_ef(P - 1)
            if P + 1 < NP:
                ad_exp2(P)

        # Pair-3 drain, split by head halves: av/normalize for half 0 runs
        # under half 1's exp2 stream instead of after it.
        LP = NP - 1
        oth = oth_pool.tile([128, N], BF, tag="oth", name=f"oth{LP}")
        oth_tiles[LP] = oth
        osbs = [osb_pool.tile([128, 2, HD], BF, tag="osb", name=f"osbL{nt}")
                for nt in range(NT)]

        def drain_half(hh):
            h = 2 * LP + hh
            for mt in range(NT):
                nc.scalar.activation(
                    t_tiles[(LP, mt)][:, hh, :], t_tiles[(LP, mt)][:, hh, :],
                    AF.Exp)
            for nt in range(NT):
                onat = ps_sm.tile([128, HD + 1], F32, tag="sm", name=f"oL{hh}_{nt}")
                for mt in range(NT):
                    nc.tensor.matmul(
                        onat,
                        t_tiles[(LP, mt)][:, hh, bass.ts(nt, 128)],
                        v_sb[:, mt, h, :],
                        start=(mt == 0), stop=(mt == NT - 1))
                rc2 = rc2_pool.tile([128, 1], F32, tag="rc2", name=f"rc2L{hh}_{nt}")
                nc.vector.reciprocal(rc2, onat[:, HD:HD + 1])
                nc.vector.tensor_scalar_mul(osbs[nt][:, hh, :], onat[:, 0:HD], rc2)

        drain_half(0)
        drain_half(1)
        for nt in range(NT):
            pt = ps_sm.tile([128, 128], BF, tag="sm", name=f"ptL{nt}")
            nc.tensor.transpose(pt, osbs[nt][:, :, :], ident)
            nc.vector.tensor_copy(oth[:, bass.ts(nt, 128)], pt)
        for mt in range(NT):
            t_tiles.pop((LP, mt))

        # ---- final projection: outT[c, n] = sum_P wp[P]^T @ oTh[P] ----
        for co in range(CT):
            ps = ps_s.tile([128, N], F32, tag="s", name=f"fin{co}")
            f = fin_pool.tile([128, N], BF, tag="f")
            for half in range(2):
                cols = bass.ts(half, 512)
                for P in range(NP):
                    nc.tensor.matmul(
                        ps[:, cols], wp_sb[:, P, bass.ts(co, 128)],
                        oth_tiles[P][:, cols],
                        start=(P == 0), stop=(P == NP - 1),
                        skip_group_check=True)
                if co % 2 == 0:
                    nc.vector.tensor_copy(f[:, cols], ps[:, cols])
                else:
                    act_copy(f[:, cols], ps[:, cols])
            if co % 2 == 0:
                nc.sync.dma_start(out=outT[co * 128:(co + 1) * 128, :], in_=f)
            else:
                nc.scalar.dma_start(out=outT[co * 128:(co + 1) * 128, :], in_=f)


def _prep(inputs):
    """Host-side shard prep: slice/transpose/cast per core."""
    x = np.asarray(inputs["x"], np.float32)
    k_in = np.asarray(inputs["k_in"], np.float32)
    attn_add = np.asarray(inputs["attn_add"], np.float32)
    Wq = np.asarray(inputs["Wq"], np.float32)
    Wkv = np.asarray(inputs["Wkv"], np.float32)
    Wproj = np.asarray(inputs["Wproj"], np.float32)
    bf = ml_dtypes.bfloat16
    in_maps = []
    for core in range(8):
        b, g = core // 2, core % 2
        h0 = g * HPC * HD  # column offset of this core's heads
        in_maps.append({
            "xT": np.ascontiguousarray(x[b].T).astype(bf),
            "kT": np.ascontiguousarray(k_in[b].T).astype(bf),
            "AT": np.ascontiguousarray(attn_add[b].T).astype(bf),
            "wq": np.ascontiguousarray(Wq[:, h0:h0 + HPC * HD]).astype(bf),
            "wk": np.ascontiguousarray(Wkv[:, h0:h0 + HPC * HD]).astype(bf),
            "wv": np.ascontiguousarray(Wkv[:, C + h0:C + h0 + HPC * HD]).astype(bf),
            "wp": np.ascontiguousarray(Wproj[h0:h0 + HPC * HD, :]).astype(bf),
        })
    return in_maps


def kernel(**inputs):
    if "nc" not in _CACHE:
        _CACHE["nc"] = _build()
    nc = _CACHE["nc"]
    in_maps = _prep(inputs)
    res = run_bass_kernel_spmd(nc, in_maps, core_ids=list(range(8)))
    bproj = np.asarray(inputs["bproj"], np.float32)
    out = np.empty((B, N, C), np.float32)
    for b in range(B):
        acc = (res.results[2 * b]["outT"].astype(np.float32)
               + res.results[2 * b + 1]["outT"].astype(np.float32))
        out[b] = acc.T + bproj
    return out


# revision 46
# speedup vs baseline: 1.0224x; 1.0224x over previous
"""CrossAttention TRN2 kernel: 8-core (batch x head-group) sharded Bass/Tile implementation.

Reference computation (per batch b):
  q = x @ Wq; kv = k_in @ Wkv -> k, v   (H=16 heads, HD=64)
  attn = softmax(q k^T * HD^-0.5); attn = softmax(attn * attn_add); out = (attn @ v) @ Wproj + bproj

Sharding: core c -> batch b = c//2, heads h0 = (c%2)*8 .. +8. Each core computes a
partial (over its 8 heads) of out[b] in TRANSPOSED layout [c_out, n]; host sums the
two partials per batch, transposes back, and adds bias.

Transposed-scores pipeline (v4): scores are computed transposed (sT[m,n] = k q^T,
keys on partitions) so the attention matrix never needs a PE transpose before
attn@v. Softmax row-sums over m (the partition axis) come from PE matmuls against
an all-ones [128,128] weight, which broadcasts r1[n] across all partitions for
free (matmul cost depends only on output columns). softmax-2's normalization is
deferred through attn@v by appending a ones column to v: output column 64 of the
natural-layout [n, hd+1] product is r2[n], per-partition, so the PSUM evacuation
is a single tensor_scalar multiply by 1/r2. attn@v accumulates with M=128 (full
partition output) at half the PE cost of the [hd, n] formulation.
"""
import sys

sys.path.insert(0, "/opt/trn_rl_repo")

import numpy as np
import ml_dtypes

import concourse.bass as bass
import concourse.tile as tile
from concourse import bacc
import concourse.mybir as mybir
from concourse.bass_utils import run_bass_kernel_spmd
from concourse.masks import make_identity

B, N, C, H = 4, 1024, 1024, 16
HD = C // H          # 64
SCALE = HD ** -0.5   # 0.125
HPC = H // 2         # 8 heads per core
NT = N // 128        # 8 n-tiles
CT = C // 128        # 8 c-tiles
NP = HPC // 2        # 4 head pairs per core
BF = mybir.dt.bfloat16
F32 = mybir.dt.float32
ALU = mybir.AluOpType
AF = mybir.ActivationFunctionType

_CACHE = {}


def _build():
    nc = bacc.Bacc("TRN2", target_bir_lowering=False, debug=False, num_devices=8)
    xT = nc.declare_dram_parameter("xT", [C, N], BF, isOutput=False)
    kT = nc.declare_dram_parameter("kT", [C, N], BF, isOutput=False)
    AT = nc.declare_dram_parameter("AT", [N, N], BF, isOutput=False)
    wq = nc.declare_dram_parameter("wq", [C, HPC * HD], BF, isOutput=False)
    wk = nc.declare_dram_parameter("wk", [C, HPC * HD], BF, isOutput=False)
    wv = nc.declare_dram_parameter("wv", [C, HPC * HD], BF, isOutput=False)
    wp = nc.declare_dram_parameter("wp", [HPC * HD, C], BF, isOutput=False)
    outT = nc.declare_dram_parameter("outT", [C, N], BF, isOutput=True)

    with tile.TileContext(nc) as tc:
        _emit(nc, tc, xT, kT, AT, wq, wk, wv, wp, outT)
    nc.compile()
    return nc


def _emit(nc, tc, xT, kT, AT, wq, wk, wv, wp, outT):
    from contextlib import ExitStack

    ctx = ExitStack()
    with ctx:
        persist = ctx.enter_context(tc.tile_pool(name="persist", bufs=1))
        ph_in = ctx.enter_context(tc.tile_pool(name="ph_in", bufs=1))
        ps_s = ctx.enter_context(tc.tile_pool(name="ps_s", bufs=2, space="PSUM"))
        ps_sm = ctx.enter_context(tc.tile_pool(name="ps_sm", bufs=4, space="PSUM"))
        e1_pool = ctx.enter_context(tc.tile_pool(name="e1p", bufs=3))
        t_pool = ctx.enter_context(tc.tile_pool(name="tp", bufs=17))
        rc1_pool = ctx.enter_context(tc.tile_pool(name="rc1", bufs=2))
        rc2_pool = ctx.enter_context(tc.tile_pool(name="rc2", bufs=8))
        osb_pool = ctx.enter_context(tc.tile_pool(name="osb", bufs=8))
        oth_pool = ctx.enter_context(tc.tile_pool(name="oth", bufs=4))
        fin_pool = ctx.enter_context(tc.tile_pool(name="fin", bufs=2))

        ident = persist.tile([128, 128], BF)
        make_identity(nc, ident)
        ones = persist.tile([128, 128], BF)
        nc.gpsimd.memset(ones, 1.0)

        a_sb = persist.tile([128, NT, N], BF)      # A^T tiles: [m-chunk, n]
        qTh = persist.tile([128, NP, N], BF)       # pack p: head 2p on parts 0-63
        kTh = persist.tile([128, NP, N], BF)
        v_sb = persist.tile([128, NT, HPC, HD + 1], BF)  # per head 65 cols, col 64 = 1
        wp_sb = persist.tile([128, NP, C], BF)

        # ---- input DMAs: 4 trigger queues, first-needed first ----
        kt = ph_in.tile([128, CT, N], BF)
        wk_sb = ph_in.tile([128, CT, HPC * HD], BF)
        wv_sb = ph_in.tile([128, CT, HPC * HD], BF)
        xt = ph_in.tile([128, CT, N], BF)
        wq_sb = ph_in.tile([128, CT, HPC * HD], BF)
        kT_r = kT.rearrange("(t p) n -> p t n", p=128)
        xT_r = xT.rearrange("(t p) n -> p t n", p=128)
        AT_r = AT.rearrange("(t p) m -> p t m", p=128)
        nc.sync.dma_start(out=wk_sb, in_=wk.rearrange("(t p) m -> p t m", p=128))
        nc.scalar.dma_start(out=wq_sb, in_=wq.rearrange("(t p) m -> p t m", p=128))
        nc.gpsimd.dma_start(out=wv_sb, in_=wv.rearrange("(t p) m -> p t m", p=128))
        # Spread x/k across all three queues, low ct first (the projection
        # chains accumulate in ct order and can stream behind the DMAs).
        nc.sync.dma_start(out=kt[:, 0:3], in_=kT_r[:, 0:3])
        nc.scalar.dma_start(out=xt[:, 0:3], in_=xT_r[:, 0:3])
        nc.gpsimd.dma_start(out=kt[:, 6:8], in_=kT_r[:, 6:8])
        nc.sync.dma_start(out=xt[:, 3:6], in_=xT_r[:, 3:6])
        nc.scalar.dma_start(out=kt[:, 3:6], in_=kT_r[:, 3:6])
        nc.gpsimd.dma_start(out=xt[:, 6:8], in_=xT_r[:, 6:8])
        # A^T and Wproj ride behind the critical tensors on each queue so
        # they don't compete for bandwidth during the prologue.
        nc.sync.dma_start(out=a_sb[:, 0:4], in_=AT_r[:, 0:4])
        nc.scalar.dma_start(out=a_sb[:, 4:8], in_=AT_r[:, 4:8])
        nc.gpsimd.dma_start(out=wp_sb, in_=wp.rearrange("(t p) m -> p t m", p=128))

        nc.gpsimd.memset(v_sb[:, :, :, HD:HD + 1], 1.0)

        def act_copy(out, in_):
            nc.scalar.activation(out, in_, AF.Copy)

        def qk_chain(p, which, on_act=False):
            """One projection chain: which = 2*is_k + half."""
            is_k, half = which // 2, which % 2
            cols = bass.ts(half, 512)
            w, src_t, dst = ((wk_sb, kt, kTh) if is_k else (wq_sb, xt, qTh))
            ps = ps_s.tile([128, 512], F32, tag="s", name=f"qk{p}_{which}")
            for ct in range(CT):
                nc.tensor.matmul(
                    ps, w[:, ct, bass.ts(p, 128)], src_t[:, ct, cols],
                    start=(ct == 0), stop=(ct == CT - 1))
            (act_copy if on_act else nc.vector.tensor_copy)(dst[:, p, cols], ps)

        def qk_proj(p, on_act):
            for which in range(4):
                qk_chain(p, which, on_act)

        def v_proj_all():
            for mt in range(NT):
                v_chain(mt)

        def v_chain(mt):
            ps = ps_s.tile([128, HPC, HD], F32, tag="s", name=f"v{mt}")
            for ct in range(CT):
                nc.tensor.matmul(
                    ps, kt[:, ct, bass.ts(mt, 128)], wv_sb[:, ct, :],
                    start=(ct == 0), stop=(ct == CT - 1))
            nc.vector.tensor_copy(v_sb[:, mt, :, 0:HD], ps)

        t_tiles = {}
        e2_tiles = {}
        oth_tiles = {}
        fillers = {}

        e1_tiles = {}
        r1_tiles = {}

        def ad_block(P, mt):
            """One mt block: transposed scores -> exp1 -> t = e1 * A^T on
            DVE/Pool, plus the r1 accumulation matmuls (all-ones weight,
            broadcast over partitions) for the PREVIOUS mt block, so the
            first block of a pair can be pre-emitted into the prior pair's
            tail without holding extra r1 psum slots."""
            e1p = e1_pool.tile([128, 2, N], BF, tag="e1", name=f"e1_{P}_{mt}")
            e1_tiles[(P, mt)] = e1p
            tp = t_pool.tile([128, 2, N], BF, tag="t", name=f"t_{P}_{mt}")
            t_tiles[(P, mt)] = tp
            for hh in range(2):
                off = hh * 64
                s = ps_s.tile([128, N], F32, tag="s", name=f"s{P}_{mt}_{hh}")
                for mc in range(2):
                    nc.tensor.matmul(
                        s[:, bass.ts(mc, 512)],
                        kTh[off:off + 64, P, bass.ts(mt, 128)],
                        qTh[off:off + 64, P, bass.ts(mc, 512)],
                        start=True, stop=True)
                nc.scalar.activation(e1p[:, hh, :], s, AF.Exp, scale=SCALE)
            if mt > 0:
                ad_r1(P, mt - 1)
            for hh in range(2):
                eng = nc.vector if hh == 0 else nc.gpsimd
                eng.tensor_mul(tp[:, hh, :], e1p[:, hh, :], a_sb[:, mt, :])
            for fill in fillers.pop((P, mt), []):
                fill()

        def ad_r1(P, mt):
            e1p = e1_tiles.pop((P, mt))
            for hh in range(2):
                for half in range(2):
                    if mt == 0:
                        r1_tiles[(P, hh, half)] = ps_sm.tile(
                            [128, 512], F32, tag="sm", name=f"r1_{P}_{hh}_{half}")
                    nc.tensor.matmul(
                        r1_tiles[(P, hh, half)], ones,
                        e1p[:, hh, bass.ts(half, 512)],
                        start=(mt == 0), stop=(mt == NT - 1),
                        skip_group_check=True)

        def ad_tail(P):
            """Close the r1 chain, take reciprocals, apply *rc1 in place."""
            ad_r1(P, NT - 1)
            rc1 = rc1_pool.tile([128, 2, N], BF, tag="rc1", name=f"rc1_{P}")
            for hh in range(2):
                for half in range(2):
                    with nc.allow_low_precision(reason="softmax1 normalizer bf16"):
                        nc.vector.reciprocal(
                            rc1[:, hh, bass.ts(half, 512)],
                            r1_tiles.pop((P, hh, half)))
            for mt in range(NT):
                tp = t_tiles[(P, mt)]
                nc.vector.tensor_mul(tp[:, :, :], tp[:, :, :], rc1)

        def ad_exp2(P):
            """exp2, in place on the merged head-pair t tiles [128, 2048]."""
            for mt in range(NT):
                tp = t_tiles.pop((P, mt))
                e2_tiles[(P, mt)] = tp
                nc.scalar.activation(tp[:, :, :], tp[:, :, :], AF.Exp)

        def stage_ef(P):
            """attn@[v|1] (natural layout, col 64 = r2), normalize, transpose.
            osb packs both heads' [n,64] blocks side by side so one [128,128]
            transpose per nt yields the [2*64 hd, n] layout proj needs. PSUM
            tiles are packed two-up to halve ps_sm ring pressure."""
            oth = oth_pool.tile([128, N], BF, tag="oth", name=f"oth{P}")
            oth_tiles[P] = oth
            osbs = []

            def ef_tp(P, nt):
                pt = ps_sm.tile([128, 128], BF, tag="sm", name=f"pt{P}_{nt}")
                nc.tensor.transpose(pt, osbs[nt][:, :, :], ident)
                nc.vector.tensor_copy(oth[:, bass.ts(nt, 128)], pt)

            for nt in range(NT):
                osb = osb_pool.tile([128, 2, HD], BF, tag="osb", name=f"osb{P}_{nt}")
                osbs.append(osb)
                for hh in range(2):
                    h = 2 * P + hh
                    onat = ps_sm.tile([128, HD + 1], F32, tag="sm", name=f"o{h}_{nt}")
                    for mt in range(NT):
                        nc.tensor.matmul(
                            onat,
                            e2_tiles[(P, mt)][:, hh, bass.ts(nt, 128)],
                            v_sb[:, mt, h, :],
                            start=(mt == 0), stop=(mt == NT - 1))
                    rc2 = rc2_pool.tile([128, 1], F32, tag="rc2", name=f"rc2_{h}_{nt}")
                    nc.vector.reciprocal(rc2, onat[:, HD:HD + 1])
                    nc.vector.tensor_scalar_mul(osb[:, hh, :], onat[:, 0:HD], rc2)
                if nt >= 2:
                    ef_tp(P, nt - 2)
            ef_tp(P, NT - 2)
            ef_tp(P, NT - 1)
            for mt in range(NT):
                e2_tiles.pop((P, mt))

        # Pipeline: each pair's first block is pre-emitted into the previous
        # pair's tail so ACT's exp1 stream never stalls at the rc1/TT2
        # handoff. qk/v projection chains are spread as per-block PE fillers
        # so no single PE wall starves ACT; the exp2 tail window absorbs the
        # previous pair's output stage.
        fillers.update({
            (0, 2): [lambda: qk_chain(1, 0), lambda: qk_chain(1, 1)],
            (0, 3): [lambda: qk_chain(1, 2), lambda: qk_chain(1, 3)],
            (0, 4): [lambda: v_chain(0)], (0, 5): [lambda: v_chain(1)],
            (0, 6): [lambda: v_chain(2)], (0, 7): [lambda: v_chain(3)],
        })
        qk_proj(0, on_act=True)
        for mt in range(NT):
            ad_block(0, mt)
        ad_block(1, 0)
        ad_tail(0)
        for mt in range(4, NT):
            v_chain(mt)
        ad_exp2(0)
        for P in range(1, NP):
            if P + 1 < NP:
                fillers.update({
                    (P, 2): [lambda w=w, p=P + 1: qk_chain(p, w) for w in (0, 1)],
                    (P, 4): [lambda w=w, p=P + 1: qk_chain(p, w) for w in (2, 3)],
                })
            for mt in range(1, NT):
                ad_block(P, mt)
            if P + 1 < NP:
                ad_block(P + 1, 0)
            ad_tail(P)
            stage_ef(P - 1)
            if P + 1 < NP:
                ad_exp2(P)

        # Pair-3 drain, split by head halves: av/normalize for half 0 runs
        # under half 1's exp2 stream instead of after it.
        LP = NP - 1
        oth = oth_pool.tile([128, N], BF, tag="oth", name=f"oth{LP}")
        oth_tiles[LP] = oth
        osbs = [osb_pool.tile([128, 2, HD], BF, tag="osb", name=f"osbL{nt}")
                for nt in range(NT)]

        def drain_half(hh):
            h = 2 * LP + hh
            for mt in range(NT):
                nc.scalar.activation(
                    t_tiles[(LP, mt)][:, hh, :], t_tiles[(LP, mt)][:, hh, :],
                    AF.Exp)
            for nt in range(NT):
                onat = ps_sm.tile([128, HD + 1], F32, tag="sm", name=f"oL{hh}_{nt}")
                for mt in range(NT):
                    nc.tensor.matmul(
                        onat,
                        t_tiles[(LP, mt)][:, hh, bass.ts(nt, 128)],
                        v_sb[:, mt, h, :],
                        start=(mt == 0), stop=(mt == NT - 1))
                rc2 = rc2_pool.tile([128, 1], F32, tag="rc2", name=f"rc2L{hh}_{nt}")
                nc.vector.reciprocal(rc2, onat[:, HD:HD + 1])
                nc.vector.tensor_scalar_mul(osbs[nt][:, hh, :], onat[:, 0:HD], rc2)

        drain_half(0)
        drain_half(1)
        for nt in range(NT):
            pt = ps_sm.tile([128, 128], BF, tag="sm", name=f"ptL{nt}")
            nc.tensor.transpose(pt, osbs[nt][:, :, :], ident)
            nc.vector.tensor_copy(oth[:, bass.ts(nt, 128)], pt)
        for mt in range(NT):
            t_tiles.pop((LP, mt))

        # ---- final projection: outT[c, n] = sum_P wp[P]^T @ oTh[P] ----
        for co in range(CT):
            ps = ps_s.tile([128, N], F32, tag="s", name=f"fin{co}")
            for half in range(2):
                cols = bass.ts(half, 512)
                for P in range(NP):
                    nc.tensor.matmul(
                        ps[:, cols], wp_sb[:, P, bass.ts(co, 128)],
                        oth_tiles[P][:, cols],
                        start=(P == 0), stop=(P == NP - 1),
                        skip_group_check=True)
            f = fin_pool.tile([128, N], BF, tag="f")
            if co % 2 == 0:
                nc.vector.tensor_copy(f, ps)
                nc.sync.dma_start(out=outT[co * 128:(co + 1) * 128, :], in_=f)
            else:
                act_copy(f, ps)
                nc.scalar.dma_start(out=outT[co * 128:(co + 1) * 128, :], in_=f)


def _prep(inputs):
    """Host-side shard prep: slice/transpose/cast per core."""
    x = np.asarray(inputs["x"], np.float32)
    k_in = np.asarray(inputs["k_in"], np.float32)
    attn_add = np.asarray(inputs["attn_add"], np.float32)
    Wq = np.asarray(inputs["Wq"], np.float32)
    Wkv = np.asarray(inputs["Wkv"], np.float32)
    Wproj = np.asarray(inputs["Wproj"], np.float32)
    bf = ml_dtypes.bfloat16
    in_maps = []
    for core in range(8):
        b, g = core // 2, core % 2
        h0 = g * HPC * HD  # column offset of this core's heads
        in_maps.append({
            "xT": np.ascontiguousarray(x[b].T).astype(bf),
            "kT": np.ascontiguousarray(k_in[b].T).astype(bf),
            "AT": np.ascontiguousarray(attn_add[b].T).astype(bf),
            "wq": np.ascontiguousarray(Wq[:, h0:h0 + HPC * HD]).astype(bf),
            "wk": np.ascontiguousarray(Wkv[:, h0:h0 + HPC * HD]).astype(bf),
            "wv": np.ascontiguousarray(Wkv[:, C + h0:C + h0 + HPC * HD]).astype(bf),
            "wp": np.ascontiguousarray(Wproj[h0:h0 + HPC * HD, :]).astype(bf),
        })
    return in_maps


def kernel(**inputs):
    if "nc" not in _CACHE:
        _CACHE["nc"] = _build()
    nc = _CACHE["nc"]
    in_maps = _prep(inputs)
    res = run_bass_kernel_spmd(nc, in_maps, core_ids=list(range(8)))
    bproj = np.asarray(inputs["bproj"], np.float32)
    out = np.empty((B, N, C), np.float32)
    for b in range(B):
        acc = (res.results[2 * b]["outT"].astype(np.float32)
               + res.results[2 * b + 1]["outT"].astype(np.float32))
        out[b] = acc.T + bproj
    return out


# revision 47
# speedup vs baseline: 1.0254x; 1.0030x over previous
"""CrossAttention TRN2 kernel: 8-core (batch x head-group) sharded Bass/Tile implementation.

Reference computation (per batch b):
  q = x @ Wq; kv = k_in @ Wkv -> k, v   (H=16 heads, HD=64)
  attn = softmax(q k^T * HD^-0.5); attn = softmax(attn * attn_add); out = (attn @ v) @ Wproj + bproj

Sharding: core c -> batch b = c//2, heads h0 = (c%2)*8 .. +8. Each core computes a
partial (over its 8 heads) of out[b] in TRANSPOSED layout [c_out, n]; host sums the
two partials per batch, transposes back, and adds bias.

Transposed-scores pipeline (v4): scores are computed transposed (sT[m,n] = k q^T,
keys on partitions) so the attention matrix never needs a PE transpose before
attn@v. Softmax row-sums over m (the partition axis) come from PE matmuls against
an all-ones [128,128] weight, which broadcasts r1[n] across all partitions for
free (matmul cost depends only on output columns). softmax-2's normalization is
deferred through attn@v by appending a ones column to v: output column 64 of the
natural-layout [n, hd+1] product is r2[n], per-partition, so the PSUM evacuation
is a single tensor_scalar multiply by 1/r2. attn@v accumulates with M=128 (full
partition output) at half the PE cost of the [hd, n] formulation.
"""
import sys

sys.path.insert(0, "/opt/trn_rl_repo")

import numpy as np
import ml_dtypes

import concourse.bass as bass
import concourse.tile as tile
from concourse import bacc
import concourse.mybir as mybir
from concourse.bass_utils import run_bass_kernel_spmd
from concourse.masks import make_identity

B, N, C, H = 4, 1024, 1024, 16
HD = C // H          # 64
SCALE = HD ** -0.5   # 0.125
HPC = H // 2         # 8 heads per core
NT = N // 128        # 8 n-tiles
CT = C // 128        # 8 c-tiles
NP = HPC // 2        # 4 head pairs per core
BF = mybir.dt.bfloat16
F32 = mybir.dt.float32
ALU = mybir.AluOpType
AF = mybir.ActivationFunctionType

_CACHE = {}


def _build():
    nc = bacc.Bacc("TRN2", target_bir_lowering=False, debug=False, num_devices=8)
    xT = nc.declare_dram_parameter("xT", [C, N], BF, isOutput=False)
    kT = nc.declare_dram_parameter("kT", [C, N], BF, isOutput=False)
    AT = nc.declare_dram_parameter("AT", [N, N], BF, isOutput=False)
    wq = nc.declare_dram_parameter("wq", [C, HPC * HD], BF, isOutput=False)
    wk = nc.declare_dram_parameter("wk", [C, HPC * HD], BF, isOutput=False)
    wv = nc.declare_dram_parameter("wv", [C, HPC * HD], BF, isOutput=False)
    wp = nc.declare_dram_parameter("wp", [HPC * HD, C], BF, isOutput=False)
    outT = nc.declare_dram_parameter("outT", [C, N], BF, isOutput=True)

    with tile.TileContext(nc) as tc:
        _emit(nc, tc, xT, kT, AT, wq, wk, wv, wp, outT)
    nc.compile()
    return nc


def _emit(nc, tc, xT, kT, AT, wq, wk, wv, wp, outT):
    from contextlib import ExitStack

    ctx = ExitStack()
    with ctx:
        persist = ctx.enter_context(tc.tile_pool(name="persist", bufs=1))
        ph_in = ctx.enter_context(tc.tile_pool(name="ph_in", bufs=1))
        ps_s = ctx.enter_context(tc.tile_pool(name="ps_s", bufs=2, space="PSUM"))
        ps_sm = ctx.enter_context(tc.tile_pool(name="ps_sm", bufs=4, space="PSUM"))
        e1_pool = ctx.enter_context(tc.tile_pool(name="e1p", bufs=3))
        t_pool = ctx.enter_context(tc.tile_pool(name="tp", bufs=17))
        rc1_pool = ctx.enter_context(tc.tile_pool(name="rc1", bufs=2))
        rc2_pool = ctx.enter_context(tc.tile_pool(name="rc2", bufs=8))
        osb_pool = ctx.enter_context(tc.tile_pool(name="osb", bufs=8))
        oth_pool = ctx.enter_context(tc.tile_pool(name="oth", bufs=4))
        fin_pool = ctx.enter_context(tc.tile_pool(name="fin", bufs=2))

        ident = persist.tile([128, 128], BF)
        make_identity(nc, ident)
        ones = persist.tile([128, 128], BF)
        nc.gpsimd.memset(ones, 1.0)

        a_sb = persist.tile([128, NT, N], BF)      # A^T tiles: [m-chunk, n]
        qTh = persist.tile([128, NP, N], BF)       # pack p: head 2p on parts 0-63
        kTh = persist.tile([128, NP, N], BF)
        v_sb = persist.tile([128, NT, HPC, HD + 1], BF)  # per head 65 cols, col 64 = 1
        wp_sb = persist.tile([128, NP, C], BF)

        # ---- input DMAs: 4 trigger queues, first-needed first ----
        kt = ph_in.tile([128, CT, N], BF)
        wk_sb = ph_in.tile([128, CT, HPC * HD], BF)
        wv_sb = ph_in.tile([128, CT, HPC * HD], BF)
        xt = ph_in.tile([128, CT, N], BF)
        wq_sb = ph_in.tile([128, CT, HPC * HD], BF)
        kT_r = kT.rearrange("(t p) n -> p t n", p=128)
        xT_r = xT.rearrange("(t p) n -> p t n", p=128)
        AT_r = AT.rearrange("(t p) m -> p t m", p=128)
        nc.sync.dma_start(out=wk_sb, in_=wk.rearrange("(t p) m -> p t m", p=128))
        nc.scalar.dma_start(out=wq_sb, in_=wq.rearrange("(t p) m -> p t m", p=128))
        nc.gpsimd.dma_start(out=wv_sb, in_=wv.rearrange("(t p) m -> p t m", p=128))
        # Spread x/k across all three queues, low ct first (the projection
        # chains accumulate in ct order and can stream behind the DMAs).
        nc.sync.dma_start(out=kt[:, 0:3], in_=kT_r[:, 0:3])
        nc.scalar.dma_start(out=xt[:, 0:3], in_=xT_r[:, 0:3])
        nc.gpsimd.dma_start(out=kt[:, 6:8], in_=kT_r[:, 6:8])
        nc.sync.dma_start(out=xt[:, 3:6], in_=xT_r[:, 3:6])
        nc.scalar.dma_start(out=kt[:, 3:6], in_=kT_r[:, 3:6])
        nc.gpsimd.dma_start(out=xt[:, 6:8], in_=xT_r[:, 6:8])
        # A^T and Wproj ride behind the critical tensors on each queue so
        # they don't compete for bandwidth during the prologue.
        nc.sync.dma_start(out=a_sb[:, 0:4], in_=AT_r[:, 0:4])
        nc.scalar.dma_start(out=a_sb[:, 4:8], in_=AT_r[:, 4:8])
        nc.gpsimd.dma_start(out=wp_sb, in_=wp.rearrange("(t p) m -> p t m", p=128))

        nc.gpsimd.memset(v_sb[:, :, :, HD:HD + 1], 1.0)

        def act_copy(out, in_):
            nc.scalar.activation(out, in_, AF.Copy)

        def qk_chain(p, which, on_act=False):
            """One projection chain: which = 2*is_k + half."""
            is_k, half = which // 2, which % 2
            cols = bass.ts(half, 512)
            w, src_t, dst = ((wk_sb, kt, kTh) if is_k else (wq_sb, xt, qTh))
            ps = ps_s.tile([128, 512], F32, tag="s", name=f"qk{p}_{which}")
            for ct in range(CT):
                nc.tensor.matmul(
                    ps, w[:, ct, bass.ts(p, 128)], src_t[:, ct, cols],
                    start=(ct == 0), stop=(ct == CT - 1))
            (act_copy if on_act else nc.vector.tensor_copy)(dst[:, p, cols], ps)

        def qk_proj(p, on_act):
            for which in range(4):
                qk_chain(p, which, on_act)

        def v_proj_all():
            for mt in range(NT):
                v_chain(mt)

        def v_chain(mt):
            ps = ps_s.tile([128, HPC, HD], F32, tag="s", name=f"v{mt}")
            for ct in range(CT):
                nc.tensor.matmul(
                    ps, kt[:, ct, bass.ts(mt, 128)], wv_sb[:, ct, :],
                    start=(ct == 0), stop=(ct == CT - 1))
            nc.vector.tensor_copy(v_sb[:, mt, :, 0:HD], ps)

        t_tiles = {}
        e2_tiles = {}
        oth_tiles = {}
        fillers = {}

        e1_tiles = {}
        r1_tiles = {}

        def ad_block(P, mt):
            """One mt block: transposed scores -> exp1 -> t = e1 * A^T on
            DVE/Pool, plus the r1 accumulation matmuls (all-ones weight,
            broadcast over partitions) for the PREVIOUS mt block, so the
            first block of a pair can be pre-emitted into the prior pair's
            tail without holding extra r1 psum slots."""
            e1p = e1_pool.tile([128, 2, N], BF, tag="e1", name=f"e1_{P}_{mt}")
            e1_tiles[(P, mt)] = e1p
            tp = t_pool.tile([128, 2, N], BF, tag="t", name=f"t_{P}_{mt}")
            t_tiles[(P, mt)] = tp
            for hh in range(2):
                off = hh * 64
                s = ps_s.tile([128, N], F32, tag="s", name=f"s{P}_{mt}_{hh}")
                for mc in range(2):
                    nc.tensor.matmul(
                        s[:, bass.ts(mc, 512)],
                        kTh[off:off + 64, P, bass.ts(mt, 128)],
                        qTh[off:off + 64, P, bass.ts(mc, 512)],
                        start=True, stop=True)
                nc.scalar.activation(e1p[:, hh, :], s, AF.Exp, scale=SCALE)
            if mt > 0:
                ad_r1(P, mt - 1)
            for hh in range(2):
                eng = nc.vector if hh == 0 else nc.gpsimd
                eng.tensor_mul(tp[:, hh, :], e1p[:, hh, :], a_sb[:, mt, :])
            for fill in fillers.pop((P, mt), []):
                fill()

        def ad_r1(P, mt):
            e1p = e1_tiles.pop((P, mt))
            for hh in range(2):
                for half in range(2):
                    if mt == 0:
                        r1_tiles[(P, hh, half)] = ps_sm.tile(
                            [128, 512], F32, tag="sm", name=f"r1_{P}_{hh}_{half}")
                    nc.tensor.matmul(
                        r1_tiles[(P, hh, half)], ones,
                        e1p[:, hh, bass.ts(half, 512)],
                        start=(mt == 0), stop=(mt == NT - 1),
                        skip_group_check=True)

        def ad_tail(P):
            """Close the r1 chain, take reciprocals, apply *rc1 in place.
            The previous pair's output stage is interleaved per-mt so its
            DVE evacuations don't queue behind the whole TT2 stretch."""
            ad_r1(P, NT - 1)
            rc1 = rc1_pool.tile([128, 2, N], BF, tag="rc1", name=f"rc1_{P}")
            for hh in range(2):
                for half in range(2):
                    with nc.allow_low_precision(reason="softmax1 normalizer bf16"):
                        nc.vector.reciprocal(
                            rc1[:, hh, bass.ts(half, 512)],
                            r1_tiles.pop((P, hh, half)))
            if P >= 1:
                ef_begin(P - 1)
            for mt in range(NT):
                tp = t_tiles[(P, mt)]
                nc.vector.tensor_mul(tp[:, :, :], tp[:, :, :], rc1)
                if P >= 1:
                    ef_nt(P - 1, mt)
            if P >= 1:
                ef_end(P - 1)

        def ad_exp2(P):
            """exp2, in place on the merged head-pair t tiles [128, 2048]."""
            for mt in range(NT):
                tp = t_tiles.pop((P, mt))
                e2_tiles[(P, mt)] = tp
                nc.scalar.activation(tp[:, :, :], tp[:, :, :], AF.Exp)

        ef_state = {}

        def ef_begin(P):
            oth = oth_pool.tile([128, N], BF, tag="oth", name=f"oth{P}")
            oth_tiles[P] = oth
            ef_state[P] = (oth, [])

        def ef_tp(P, nt):
            oth, osbs = ef_state[P]
            pt = ps_sm.tile([128, 128], BF, tag="sm", name=f"pt{P}_{nt}")
            nc.tensor.transpose(pt, osbs[nt][:, :, :], ident)
            nc.vector.tensor_copy(oth[:, bass.ts(nt, 128)], pt)

        def ef_nt(P, nt):
            """attn@[v|1] for one n-tile (both heads): natural layout, col 64
            = r2, normalized on evacuation, transposed with lag 2."""
            oth, osbs = ef_state[P]
            osb = osb_pool.tile([128, 2, HD], BF, tag="osb", name=f"osb{P}_{nt}")
            osbs.append(osb)
            for hh in range(2):
                h = 2 * P + hh
                onat = ps_sm.tile([128, HD + 1], F32, tag="sm", name=f"o{h}_{nt}")
                for mt in range(NT):
                    nc.tensor.matmul(
                        onat,
                        e2_tiles[(P, mt)][:, hh, bass.ts(nt, 128)],
                        v_sb[:, mt, h, :],
                        start=(mt == 0), stop=(mt == NT - 1))
                rc2 = rc2_pool.tile([128, 1], F32, tag="rc2", name=f"rc2_{h}_{nt}")
                nc.vector.reciprocal(rc2, onat[:, HD:HD + 1])
                nc.vector.tensor_scalar_mul(osb[:, hh, :], onat[:, 0:HD], rc2)
            if nt >= 2:
                ef_tp(P, nt - 2)

        def ef_end(P):
            ef_tp(P, NT - 2)
            ef_tp(P, NT - 1)
            for mt in range(NT):
                e2_tiles.pop((P, mt))

        # Pipeline: each pair's first block is pre-emitted into the previous
        # pair's tail so ACT's exp1 stream never stalls at the rc1/TT2
        # handoff. qk/v projection chains are spread as per-block PE fillers
        # so no single PE wall starves ACT; the exp2 tail window absorbs the
        # previous pair's output stage.
        fillers.update({
            (0, 2): [lambda: qk_chain(1, 0), lambda: qk_chain(1, 1)],
            (0, 3): [lambda: qk_chain(1, 2), lambda: qk_chain(1, 3)],
            (0, 4): [lambda: v_chain(0)], (0, 5): [lambda: v_chain(1)],
            (0, 6): [lambda: v_chain(2)], (0, 7): [lambda: v_chain(3)],
        })
        qk_proj(0, on_act=True)
        for mt in range(NT):
            ad_block(0, mt)
        ad_block(1, 0)
        ad_tail(0)
        for mt in range(4, NT):
            v_chain(mt)
        ad_exp2(0)
        for P in range(1, NP):
            if P + 1 < NP:
                fillers.update({
                    (P, 2): [lambda w=w, p=P + 1: qk_chain(p, w) for w in (0, 1)],
                    (P, 4): [lambda w=w, p=P + 1: qk_chain(p, w) for w in (2, 3)],
                })
            for mt in range(1, NT):
                ad_block(P, mt)
            if P + 1 < NP:
                ad_block(P + 1, 0)
            ad_tail(P)
            if P + 1 < NP:
                ad_exp2(P)

        # Pair-3 drain, split by head halves: av/normalize for half 0 runs
        # under half 1's exp2 stream instead of after it.
        LP = NP - 1
        oth = oth_pool.tile([128, N], BF, tag="oth", name=f"oth{LP}")
        oth_tiles[LP] = oth
        osbs = [osb_pool.tile([128, 2, HD], BF, tag="osb", name=f"osbL{nt}")
                for nt in range(NT)]

        def drain_half(hh):
            h = 2 * LP + hh
            for mt in range(NT):
                nc.scalar.activation(
                    t_tiles[(LP, mt)][:, hh, :], t_tiles[(LP, mt)][:, hh, :],
                    AF.Exp)
            for nt in range(NT):
                onat = ps_sm.tile([128, HD + 1], F32, tag="sm", name=f"oL{hh}_{nt}")
                for mt in range(NT):
                    nc.tensor.matmul(
                        onat,
                        t_tiles[(LP, mt)][:, hh, bass.ts(nt, 128)],
                        v_sb[:, mt, h, :],
                        start=(mt == 0), stop=(mt == NT - 1))
                rc2 = rc2_pool.tile([128, 1], F32, tag="rc2", name=f"rc2L{hh}_{nt}")
                nc.vector.reciprocal(rc2, onat[:, HD:HD + 1])
                nc.vector.tensor_scalar_mul(osbs[nt][:, hh, :], onat[:, 0:HD], rc2)

        drain_half(0)
        drain_half(1)
        for nt in range(NT):
            pt = ps_sm.tile([128, 128], BF, tag="sm", name=f"ptL{nt}")
            nc.tensor.transpose(pt, osbs[nt][:, :, :], ident)
            nc.vector.tensor_copy(oth[:, bass.ts(nt, 128)], pt)
        for mt in range(NT):
            t_tiles.pop((LP, mt))

        # ---- final projection: outT[c, n] = sum_P wp[P]^T @ oTh[P] ----
        for co in range(CT):
            ps = ps_s.tile([128, N], F32, tag="s", name=f"fin{co}")
            for half in range(2):
                cols = bass.ts(half, 512)
                for P in range(NP):
                    nc.tensor.matmul(
                        ps[:, cols], wp_sb[:, P, bass.ts(co, 128)],
                        oth_tiles[P][:, cols],
                        start=(P == 0), stop=(P == NP - 1),
                        skip_group_check=True)
            f = fin_pool.tile([128, N], BF, tag="f")
            if co % 2 == 0:
                nc.vector.tensor_copy(f, ps)
                nc.sync.dma_start(out=outT[co * 128:(co + 1) * 128, :], in_=f)
            else:
                act_copy(f, ps)
                nc.scalar.dma_start(out=outT[co * 128:(co + 1) * 128, :], in_=f)


def _prep(inputs):
    """Host-side shard prep: slice/transpose/cast per core."""
    x = np.asarray(inputs["x"], np.float32)
    k_in = np.asarray(inputs["k_in"], np.float32)
    attn_add = np.asarray(inputs["attn_add"], np.float32)
    Wq = np.asarray(inputs["Wq"], np.float32)
    Wkv = np.asarray(inputs["Wkv"], np.float32)
    Wproj = np.asarray(inputs["Wproj"], np.float32)
    bf = ml_dtypes.bfloat16
    in_maps = []
    for core in range(8):
        b, g = core // 2, core % 2
        h0 = g * HPC * HD  # column offset of this core's heads
        in_maps.append({
            "xT": np.ascontiguousarray(x[b].T).astype(bf),
            "kT": np.ascontiguousarray(k_in[b].T).astype(bf),
            "AT": np.ascontiguousarray(attn_add[b].T).astype(bf),
            "wq": np.ascontiguousarray(Wq[:, h0:h0 + HPC * HD]).astype(bf),
            "wk": np.ascontiguousarray(Wkv[:, h0:h0 + HPC * HD]).astype(bf),
            "wv": np.ascontiguousarray(Wkv[:, C + h0:C + h0 + HPC * HD]).astype(bf),
            "wp": np.ascontiguousarray(Wproj[h0:h0 + HPC * HD, :]).astype(bf),
        })
    return in_maps


def kernel(**inputs):
    if "nc" not in _CACHE:
        _CACHE["nc"] = _build()
    nc = _CACHE["nc"]
    in_maps = _prep(inputs)
    res = run_bass_kernel_spmd(nc, in_maps, core_ids=list(range(8)))
    bproj = np.asarray(inputs["bproj"], np.float32)
    out = np.empty((B, N, C), np.float32)
    for b in range(B):
        acc = (res.results[2 * b]["outT"].astype(np.float32)
               + res.results[2 * b + 1]["outT"].astype(np.float32))
        out[b] = acc.T + bproj
    return out


# revision 50
# speedup vs baseline: 1.0276x; 1.0021x over previous
"""CrossAttention TRN2 kernel: 8-core (batch x head-group) sharded Bass/Tile implementation.

Reference computation (per batch b):
  q = x @ Wq; kv = k_in @ Wkv -> k, v   (H=16 heads, HD=64)
  attn = softmax(q k^T * HD^-0.5); attn = softmax(attn * attn_add); out = (attn @ v) @ Wproj + bproj

Sharding: core c -> batch b = c//2, heads h0 = (c%2)*8 .. +8. Each core computes a
partial (over its 8 heads) of out[b] in TRANSPOSED layout [c_out, n]; host sums the
two partials per batch, transposes back, and adds bias.

Transposed-scores pipeline (v4): scores are computed transposed (sT[m,n] = k q^T,
keys on partitions) so the attention matrix never needs a PE transpose before
attn@v. Softmax row-sums over m (the partition axis) come from PE matmuls against
an all-ones [128,128] weight, which broadcasts r1[n] across all partitions for
free (matmul cost depends only on output columns). softmax-2's normalization is
deferred through attn@v by appending a ones column to v: output column 64 of the
natural-layout [n, hd+1] product is r2[n], per-partition, so the PSUM evacuation
is a single tensor_scalar multiply by 1/r2. attn@v accumulates with M=128 (full
partition output) at half the PE cost of the [hd, n] formulation.
"""
import sys

sys.path.insert(0, "/opt/trn_rl_repo")

import numpy as np
import ml_dtypes

import concourse.bass as bass
import concourse.tile as tile
from concourse import bacc
import concourse.mybir as mybir
from concourse.bass_utils import run_bass_kernel_spmd
from concourse.masks import make_identity

B, N, C, H = 4, 1024, 1024, 16
HD = C // H          # 64
SCALE = HD ** -0.5   # 0.125
HPC = H // 2         # 8 heads per core
NT = N // 128        # 8 n-tiles
CT = C // 128        # 8 c-tiles
NP = HPC // 2        # 4 head pairs per core
BF = mybir.dt.bfloat16
F32 = mybir.dt.float32
ALU = mybir.AluOpType
AF = mybir.ActivationFunctionType

_CACHE = {}


def _build():
    nc = bacc.Bacc("TRN2", target_bir_lowering=False, debug=False, num_devices=8)
    xT = nc.declare_dram_parameter("xT", [C, N], BF, isOutput=False)
    kT = nc.declare_dram_parameter("kT", [C, N], BF, isOutput=False)
    AT = nc.declare_dram_parameter("AT", [N, N], BF, isOutput=False)
    wq = nc.declare_dram_parameter("wq", [C, HPC * HD], BF, isOutput=False)
    wk = nc.declare_dram_parameter("wk", [C, HPC * HD], BF, isOutput=False)
    wv = nc.declare_dram_parameter("wv", [C, HPC * HD], BF, isOutput=False)
    wp = nc.declare_dram_parameter("wp", [HPC * HD, C], BF, isOutput=False)
    outT = nc.declare_dram_parameter("outT", [C, N], BF, isOutput=True)

    with tile.TileContext(nc) as tc:
        _emit(nc, tc, xT, kT, AT, wq, wk, wv, wp, outT)
    nc.compile()
    return nc


def _emit(nc, tc, xT, kT, AT, wq, wk, wv, wp, outT):
    from contextlib import ExitStack

    ctx = ExitStack()
    with ctx:
        persist = ctx.enter_context(tc.tile_pool(name="persist", bufs=1))
        ph_in = ctx.enter_context(tc.tile_pool(name="ph_in", bufs=1))
        ps_s = ctx.enter_context(tc.tile_pool(name="ps_s", bufs=2, space="PSUM"))
        ps_sm = ctx.enter_context(tc.tile_pool(name="ps_sm", bufs=4, space="PSUM"))
        e1_pool = ctx.enter_context(tc.tile_pool(name="e1p", bufs=3))
        t_pool = ctx.enter_context(tc.tile_pool(name="tp", bufs=17))
        rc1_pool = ctx.enter_context(tc.tile_pool(name="rc1", bufs=2))
        rc2_pool = ctx.enter_context(tc.tile_pool(name="rc2", bufs=8))
        osb_pool = ctx.enter_context(tc.tile_pool(name="osb", bufs=8))
        oth_pool = ctx.enter_context(tc.tile_pool(name="oth", bufs=4))
        fin_pool = ctx.enter_context(tc.tile_pool(name="fin", bufs=2))

        ident = persist.tile([128, 128], BF)
        make_identity(nc, ident)
        ones = persist.tile([128, 128], BF)
        nc.gpsimd.memset(ones, 1.0)

        a_sb = persist.tile([128, NT, N], BF)      # A^T tiles: [m-chunk, n]
        qTh = persist.tile([128, NP, N], BF)       # pack p: head 2p on parts 0-63
        kTh = persist.tile([128, NP, N], BF)
        v_sb = persist.tile([128, NT, HPC, HD + 1], BF)  # per head 65 cols, col 64 = 1
        wp_sb = persist.tile([128, NP, C], BF)

        # ---- input DMAs: 4 trigger queues, first-needed first ----
        kt = ph_in.tile([128, CT, N], BF)
        wk_sb = ph_in.tile([128, CT, HPC * HD], BF)
        wv_sb = ph_in.tile([128, CT, HPC * HD], BF)
        xt = ph_in.tile([128, CT, N], BF)
        wq_sb = ph_in.tile([128, CT, HPC * HD], BF)
        kT_r = kT.rearrange("(t p) n -> p t n", p=128)
        xT_r = xT.rearrange("(t p) n -> p t n", p=128)
        AT_r = AT.rearrange("(t p) m -> p t m", p=128)
        nc.sync.dma_start(out=wk_sb, in_=wk.rearrange("(t p) m -> p t m", p=128))
        nc.scalar.dma_start(out=wq_sb, in_=wq.rearrange("(t p) m -> p t m", p=128))
        nc.gpsimd.dma_start(out=wv_sb, in_=wv.rearrange("(t p) m -> p t m", p=128))
        # Spread x/k across all three queues, low ct first (the projection
        # chains accumulate in ct order and can stream behind the DMAs).
        nc.sync.dma_start(out=kt[:, 0:3], in_=kT_r[:, 0:3])
        nc.scalar.dma_start(out=xt[:, 0:3], in_=xT_r[:, 0:3])
        nc.gpsimd.dma_start(out=kt[:, 6:8], in_=kT_r[:, 6:8])
        nc.sync.dma_start(out=xt[:, 3:6], in_=xT_r[:, 3:6])
        nc.scalar.dma_start(out=kt[:, 3:6], in_=kT_r[:, 3:6])
        nc.gpsimd.dma_start(out=xt[:, 6:8], in_=xT_r[:, 6:8])
        # A^T and Wproj ride behind the critical tensors on each queue so
        # they don't compete for bandwidth during the prologue.
        nc.sync.dma_start(out=a_sb[:, 0:4], in_=AT_r[:, 0:4])
        nc.scalar.dma_start(out=a_sb[:, 4:8], in_=AT_r[:, 4:8])
        nc.gpsimd.dma_start(out=wp_sb, in_=wp.rearrange("(t p) m -> p t m", p=128))

        nc.gpsimd.memset(v_sb[:, :, :, HD:HD + 1], 1.0)

        def act_copy(out, in_):
            nc.scalar.activation(out, in_, AF.Copy)

        def qk_chain(p, which, on_act=False):
            """One projection chain: which = 2*is_k + half."""
            is_k, half = which // 2, which % 2
            cols = bass.ts(half, 512)
            w, src_t, dst = ((wk_sb, kt, kTh) if is_k else (wq_sb, xt, qTh))
            ps = ps_s.tile([128, 512], F32, tag="s", name=f"qk{p}_{which}")
            for ct in range(CT):
                nc.tensor.matmul(
                    ps, w[:, ct, bass.ts(p, 128)], src_t[:, ct, cols],
                    start=(ct == 0), stop=(ct == CT - 1))
            (act_copy if on_act else nc.vector.tensor_copy)(dst[:, p, cols], ps)

        def qk_proj(p, on_act):
            for which in range(4):
                qk_chain(p, which, on_act)

        def v_proj_all():
            for mt in range(NT):
                v_chain(mt)

        def v_chain(mt):
            ps = ps_s.tile([128, HPC, HD], F32, tag="s", name=f"v{mt}")
            for ct in range(CT):
                nc.tensor.matmul(
                    ps, kt[:, ct, bass.ts(mt, 128)], wv_sb[:, ct, :],
                    start=(ct == 0), stop=(ct == CT - 1))
            nc.vector.tensor_copy(v_sb[:, mt, :, 0:HD], ps)

        t_tiles = {}
        e2_tiles = {}
        oth_tiles = {}
        fillers = {}

        e1_tiles = {}
        r1_tiles = {}

        def ad_block(P, mt):
            """One mt block: transposed scores -> exp1 -> t = e1 * A^T on
            DVE/Pool, plus the r1 accumulation matmuls (all-ones weight,
            broadcast over partitions) for the PREVIOUS mt block, so the
            first block of a pair can be pre-emitted into the prior pair's
            tail without holding extra r1 psum slots."""
            e1p = e1_pool.tile([128, 2, N], BF, tag="e1", name=f"e1_{P}_{mt}")
            e1_tiles[(P, mt)] = e1p
            tp = t_pool.tile([128, 2, N], BF, tag="t", name=f"t_{P}_{mt}")
            t_tiles[(P, mt)] = tp
            for hh in range(2):
                off = hh * 64
                s = ps_s.tile([128, N], F32, tag="s", name=f"s{P}_{mt}_{hh}")
                for mc in range(2):
                    nc.tensor.matmul(
                        s[:, bass.ts(mc, 512)],
                        kTh[off:off + 64, P, bass.ts(mt, 128)],
                        qTh[off:off + 64, P, bass.ts(mc, 512)],
                        start=True, stop=True)
                nc.scalar.activation(e1p[:, hh, :], s, AF.Exp, scale=SCALE)
            if mt > 0:
                ad_r1(P, mt - 1)
            for hh in range(2):
                eng = nc.vector if hh == 0 else nc.gpsimd
                eng.tensor_mul(tp[:, hh, :], e1p[:, hh, :], a_sb[:, mt, :])
            for fill in fillers.pop((P, mt), []):
                fill()

        def ad_r1(P, mt):
            e1p = e1_tiles.pop((P, mt))
            for hh in range(2):
                for half in range(2):
                    if mt == 0:
                        r1_tiles[(P, hh, half)] = ps_sm.tile(
                            [128, 512], F32, tag="sm", name=f"r1_{P}_{hh}_{half}")
                    nc.tensor.matmul(
                        r1_tiles[(P, hh, half)], ones,
                        e1p[:, hh, bass.ts(half, 512)],
                        start=(mt == 0), stop=(mt == NT - 1),
                        skip_group_check=True)

        def ad_tail(P):
            """Close the r1 chain, take reciprocals, apply *rc1 in place.
            The previous pair's output stage is interleaved per-mt so its
            DVE evacuations don't queue behind the whole TT2 stretch."""
            ad_r1(P, NT - 1)
            rc1 = rc1_pool.tile([128, 2, N], BF, tag="rc1", name=f"rc1_{P}")
            for hh in range(2):
                for half in range(2):
                    with nc.allow_low_precision(reason="softmax1 normalizer bf16"):
                        nc.vector.reciprocal(
                            rc1[:, hh, bass.ts(half, 512)],
                            r1_tiles.pop((P, hh, half)))
            if P >= 1:
                ef_begin(P - 1)
            for mt in range(NT):
                tp = t_tiles[(P, mt)]
                nc.vector.tensor_mul(tp[:, :, :], tp[:, :, :], rc1)
                if P >= 1:
                    ef_nt(P - 1, mt)
            if P >= 1:
                ef_end(P - 1)

        def ad_exp2(P):
            """exp2, in place on the merged head-pair t tiles [128, 2048]."""
            for mt in range(NT):
                tp = t_tiles.pop((P, mt))
                e2_tiles[(P, mt)] = tp
                nc.scalar.activation(tp[:, :, :], tp[:, :, :], AF.Exp)

        ef_state = {}

        def ef_begin(P):
            oth = oth_pool.tile([128, N], BF, tag="oth", name=f"oth{P}")
            oth_tiles[P] = oth
            ef_state[P] = (oth, [])

        def ef_tp(P, nt):
            oth, osbs = ef_state[P]
            pt = ps_sm.tile([128, 128], BF, tag="sm", name=f"pt{P}_{nt}")
            nc.tensor.transpose(pt, osbs[nt][:, :, :], ident)
            nc.vector.tensor_copy(oth[:, bass.ts(nt, 128)], pt)

        def ef_nt(P, nt):
            """attn@[v|1] for one n-tile (both heads): natural layout, col 64
            = r2, normalized on evacuation, transposed with lag 2."""
            oth, osbs = ef_state[P]
            osb = osb_pool.tile([128, 2, HD], BF, tag="osb", name=f"osb{P}_{nt}")
            osbs.append(osb)
            for hh in range(2):
                h = 2 * P + hh
                onat = ps_sm.tile([128, HD + 1], F32, tag="sm", name=f"o{h}_{nt}")
                for mt in range(NT):
                    nc.tensor.matmul(
                        onat,
                        e2_tiles[(P, mt)][:, hh, bass.ts(nt, 128)],
                        v_sb[:, mt, h, :],
                        start=(mt == 0), stop=(mt == NT - 1))
                rc2 = rc2_pool.tile([128, 1], F32, tag="rc2", name=f"rc2_{h}_{nt}")
                nc.vector.reciprocal(rc2, onat[:, HD:HD + 1])
                nc.vector.tensor_scalar_mul(osb[:, hh, :], onat[:, 0:HD], rc2)
            if nt >= 2:
                ef_tp(P, nt - 2)

        def ef_end(P):
            ef_tp(P, NT - 2)
            ef_tp(P, NT - 1)
            for mt in range(NT):
                e2_tiles.pop((P, mt))

        # Pipeline: each pair's first block is pre-emitted into the previous
        # pair's tail so ACT's exp1 stream never stalls at the rc1/TT2
        # handoff. qk/v projection chains are spread as per-block PE fillers
        # so no single PE wall starves ACT; the exp2 tail window absorbs the
        # previous pair's output stage.
        fillers.update({
            (0, 2): [lambda: qk_chain(1, 0), lambda: qk_chain(1, 1)],
            (0, 3): [lambda: qk_chain(1, 2), lambda: qk_chain(1, 3)],
            (0, 4): [lambda: v_chain(0)], (0, 5): [lambda: v_chain(1)],
            (0, 6): [lambda: v_chain(2)], (0, 7): [lambda: v_chain(3)],
        })
        qk_proj(0, on_act=True)
        for mt in range(NT):
            ad_block(0, mt)
        ad_block(1, 0)
        ad_tail(0)
        for mt in range(4, NT):
            v_chain(mt)
        ad_exp2(0)
        for P in range(1, NP):
            if P + 1 < NP:
                fillers.update({
                    (P, 2): [lambda w=w, p=P + 1: qk_chain(p, w) for w in (0, 1)],
                    (P, 4): [lambda w=w, p=P + 1: qk_chain(p, w) for w in (2, 3)],
                })
            for mt in range(1, NT):
                ad_block(P, mt)
            if P + 1 < NP:
                ad_block(P + 1, 0)
            ad_tail(P)
            if P + 1 < NP:
                ad_exp2(P)

        # Pair-3 drain, split by head halves: av/normalize for half 0 runs
        # under half 1's exp2 stream instead of after it.
        LP = NP - 1
        oth = oth_pool.tile([128, N], BF, tag="oth", name=f"oth{LP}")
        oth_tiles[LP] = oth
        osbs = [osb_pool.tile([128, 2, HD], BF, tag="osb", name=f"osbL{nt}")
                for nt in range(NT)]

        def drain_half(hh, tp_lag=None):
            h = 2 * LP + hh
            for mt in range(NT):
                nc.scalar.activation(
                    t_tiles[(LP, mt)][:, hh, :], t_tiles[(LP, mt)][:, hh, :],
                    AF.Exp)
            for nt in range(NT):
                onat = ps_sm.tile([128, HD + 1], F32, tag="sm", name=f"oL{hh}_{nt}")
                for mt in range(NT):
                    nc.tensor.matmul(
                        onat,
                        t_tiles[(LP, mt)][:, hh, bass.ts(nt, 128)],
                        v_sb[:, mt, h, :],
                        start=(mt == 0), stop=(mt == NT - 1))
                rc2 = rc2_pool.tile([128, 1], F32, tag="rc2", name=f"rc2L{hh}_{nt}")
                nc.vector.reciprocal(rc2, onat[:, HD:HD + 1])
                nc.vector.tensor_scalar_mul(osbs[nt][:, hh, :], onat[:, 0:HD], rc2)
                if tp_lag is not None and nt >= 2:
                    tp_lag(nt - 2)

        def drain_tp(nt):
            pt = ps_sm.tile([128, 128], BF, tag="sm", name=f"ptL{nt}")
            nc.tensor.transpose(pt, osbs[nt][:, :, :], ident)
            nc.vector.tensor_copy(oth[:, bass.ts(nt, 128)], pt)

        drain_half(0)
        drain_half(1, tp_lag=drain_tp)
        drain_tp(NT - 2)
        drain_tp(NT - 1)
        for mt in range(NT):
            t_tiles.pop((LP, mt))

        # ---- final projection: outT[c, n] = sum_P wp[P]^T @ oTh[P] ----
        for co in range(CT):
            ps = ps_s.tile([128, N], F32, tag="s", name=f"fin{co}")
            for half in range(2):
                cols = bass.ts(half, 512)
                for P in range(NP):
                    nc.tensor.matmul(
                        ps[:, cols], wp_sb[:, P, bass.ts(co, 128)],
                        oth_tiles[P][:, cols],
                        start=(P == 0), stop=(P == NP - 1),
                        skip_group_check=True)
            f = fin_pool.tile([128, N], BF, tag="f")
            if co % 2 == 0:
                nc.vector.tensor_copy(f, ps)
                nc.sync.dma_start(out=outT[co * 128:(co + 1) * 128, :], in_=f)
            else:
                act_copy(f, ps)
                nc.scalar.dma_start(out=outT[co * 128:(co + 1) * 128, :], in_=f)


def _prep(inputs):
    """Host-side shard prep: slice/transpose/cast per core."""
    x = np.asarray(inputs["x"], np.float32)
    k_in = np.asarray(inputs["k_in"], np.float32)
    attn_add = np.asarray(inputs["attn_add"], np.float32)
    Wq = np.asarray(inputs["Wq"], np.float32)
    Wkv = np.asarray(inputs["Wkv"], np.float32)
    Wproj = np.asarray(inputs["Wproj"], np.float32)
    bf = ml_dtypes.bfloat16
    in_maps = []
    for core in range(8):
        b, g = core // 2, core % 2
        h0 = g * HPC * HD  # column offset of this core's heads
        in_maps.append({
            "xT": np.ascontiguousarray(x[b].T).astype(bf),
            "kT": np.ascontiguousarray(k_in[b].T).astype(bf),
            "AT": np.ascontiguousarray(attn_add[b].T).astype(bf),
            "wq": np.ascontiguousarray(Wq[:, h0:h0 + HPC * HD]).astype(bf),
            "wk": np.ascontiguousarray(Wkv[:, h0:h0 + HPC * HD]).astype(bf),
            "wv": np.ascontiguousarray(Wkv[:, C + h0:C + h0 + HPC * HD]).astype(bf),
            "wp": np.ascontiguousarray(Wproj[h0:h0 + HPC * HD, :]).astype(bf),
        })
    return in_maps


def kernel(**inputs):
    if "nc" not in _CACHE:
        _CACHE["nc"] = _build()
    nc = _CACHE["nc"]
    in_maps = _prep(inputs)
    res = run_bass_kernel_spmd(nc, in_maps, core_ids=list(range(8)))
    bproj = np.asarray(inputs["bproj"], np.float32)
    out = np.empty((B, N, C), np.float32)
    for b in range(B):
        acc = (res.results[2 * b]["outT"].astype(np.float32)
               + res.results[2 * b + 1]["outT"].astype(np.float32))
        out[b] = acc.T + bproj
    return out


# revision 51
# speedup vs baseline: 1.0279x; 1.0003x over previous
"""CrossAttention TRN2 kernel: 8-core (batch x head-group) sharded Bass/Tile implementation.

Reference computation (per batch b):
  q = x @ Wq; kv = k_in @ Wkv -> k, v   (H=16 heads, HD=64)
  attn = softmax(q k^T * HD^-0.5); attn = softmax(attn * attn_add); out = (attn @ v) @ Wproj + bproj

Sharding: core c -> batch b = c//2, heads h0 = (c%2)*8 .. +8. Each core computes a
partial (over its 8 heads) of out[b] in TRANSPOSED layout [c_out, n]; host sums the
two partials per batch, transposes back, and adds bias.

Transposed-scores pipeline (v4): scores are computed transposed (sT[m,n] = k q^T,
keys on partitions) so the attention matrix never needs a PE transpose before
attn@v. Softmax row-sums over m (the partition axis) come from PE matmuls against
an all-ones [128,128] weight, which broadcasts r1[n] across all partitions for
free (matmul cost depends only on output columns). softmax-2's normalization is
deferred through attn@v by appending a ones column to v: output column 64 of the
natural-layout [n, hd+1] product is r2[n], per-partition, so the PSUM evacuation
is a single tensor_scalar multiply by 1/r2. attn@v accumulates with M=128 (full
partition output) at half the PE cost of the [hd, n] formulation.
"""
import sys

sys.path.insert(0, "/opt/trn_rl_repo")

import numpy as np
import ml_dtypes

import concourse.bass as bass
import concourse.tile as tile
from concourse import bacc
import concourse.mybir as mybir
from concourse.bass_utils import run_bass_kernel_spmd
from concourse.masks import make_identity

B, N, C, H = 4, 1024, 1024, 16
HD = C // H          # 64
SCALE = HD ** -0.5   # 0.125
HPC = H // 2         # 8 heads per core
NT = N // 128        # 8 n-tiles
CT = C // 128        # 8 c-tiles
NP = HPC // 2        # 4 head pairs per core
BF = mybir.dt.bfloat16
F32 = mybir.dt.float32
ALU = mybir.AluOpType
AF = mybir.ActivationFunctionType

_CACHE = {}


def _build():
    nc = bacc.Bacc("TRN2", target_bir_lowering=False, debug=False, num_devices=8)
    xT = nc.declare_dram_parameter("xT", [C, N], BF, isOutput=False)
    kT = nc.declare_dram_parameter("kT", [C, N], BF, isOutput=False)
    AT = nc.declare_dram_parameter("AT", [N, N], BF, isOutput=False)
    wq = nc.declare_dram_parameter("wq", [C, HPC * HD], BF, isOutput=False)
    wk = nc.declare_dram_parameter("wk", [C, HPC * HD], BF, isOutput=False)
    wv = nc.declare_dram_parameter("wv", [C, HPC * HD], BF, isOutput=False)
    wp = nc.declare_dram_parameter("wp", [HPC * HD, C], BF, isOutput=False)
    outT = nc.declare_dram_parameter("outT", [C, N], BF, isOutput=True)

    with tile.TileContext(nc) as tc:
        _emit(nc, tc, xT, kT, AT, wq, wk, wv, wp, outT)
    nc.compile()
    return nc


def _emit(nc, tc, xT, kT, AT, wq, wk, wv, wp, outT):
    from contextlib import ExitStack

    ctx = ExitStack()
    with ctx:
        persist = ctx.enter_context(tc.tile_pool(name="persist", bufs=1))
        ph_in = ctx.enter_context(tc.tile_pool(name="ph_in", bufs=1))
        ps_s = ctx.enter_context(tc.tile_pool(name="ps_s", bufs=2, space="PSUM"))
        ps_sm = ctx.enter_context(tc.tile_pool(name="ps_sm", bufs=4, space="PSUM"))
        e1_pool = ctx.enter_context(tc.tile_pool(name="e1p", bufs=3))
        t_pool = ctx.enter_context(tc.tile_pool(name="tp", bufs=17))
        rc1_pool = ctx.enter_context(tc.tile_pool(name="rc1", bufs=2))
        rc2_pool = ctx.enter_context(tc.tile_pool(name="rc2", bufs=8))
        osb_pool = ctx.enter_context(tc.tile_pool(name="osb", bufs=8))
        oth_pool = ctx.enter_context(tc.tile_pool(name="oth", bufs=4))
        fin_pool = ctx.enter_context(tc.tile_pool(name="fin", bufs=2))

        ident = persist.tile([128, 128], BF)
        make_identity(nc, ident)
        ones = persist.tile([128, 128], BF)
        nc.gpsimd.memset(ones, 1.0)

        a_sb = persist.tile([128, NT, N], BF)      # A^T tiles: [m-chunk, n]
        qTh = persist.tile([128, NP, N], BF)       # pack p: head 2p on parts 0-63
        kTh = persist.tile([128, NP, N], BF)
        v_sb = persist.tile([128, NT, HPC, HD + 1], BF)  # per head 65 cols, col 64 = 1
        wp_sb = persist.tile([128, NP, C], BF)

        # ---- input DMAs: 4 trigger queues, first-needed first ----
        kt = ph_in.tile([128, CT, N], BF)
        wk_sb = ph_in.tile([128, CT, HPC * HD], BF)
        wv_sb = ph_in.tile([128, CT, HPC * HD], BF)
        xt = ph_in.tile([128, CT, N], BF)
        wq_sb = ph_in.tile([128, CT, HPC * HD], BF)
        kT_r = kT.rearrange("(t p) n -> p t n", p=128)
        xT_r = xT.rearrange("(t p) n -> p t n", p=128)
        AT_r = AT.rearrange("(t p) m -> p t m", p=128)
        nc.sync.dma_start(out=wk_sb, in_=wk.rearrange("(t p) m -> p t m", p=128))
        nc.scalar.dma_start(out=wq_sb, in_=wq.rearrange("(t p) m -> p t m", p=128))
        nc.gpsimd.dma_start(out=wv_sb, in_=wv.rearrange("(t p) m -> p t m", p=128))
        # Spread x/k across all three queues, low ct first (the projection
        # chains accumulate in ct order and can stream behind the DMAs).
        nc.sync.dma_start(out=kt[:, 0:3], in_=kT_r[:, 0:3])
        nc.scalar.dma_start(out=xt[:, 0:3], in_=xT_r[:, 0:3])
        nc.gpsimd.dma_start(out=kt[:, 6:8], in_=kT_r[:, 6:8])
        nc.sync.dma_start(out=xt[:, 3:6], in_=xT_r[:, 3:6])
        nc.scalar.dma_start(out=kt[:, 3:6], in_=kT_r[:, 3:6])
        nc.gpsimd.dma_start(out=xt[:, 6:8], in_=xT_r[:, 6:8])
        # A^T and Wproj ride behind the critical tensors on each queue so
        # they don't compete for bandwidth during the prologue.
        nc.sync.dma_start(out=a_sb[:, 0:4], in_=AT_r[:, 0:4])
        nc.scalar.dma_start(out=a_sb[:, 4:8], in_=AT_r[:, 4:8])
        nc.gpsimd.dma_start(out=wp_sb, in_=wp.rearrange("(t p) m -> p t m", p=128))

        nc.gpsimd.memset(v_sb[:, :, :, HD:HD + 1], 1.0)

        def act_copy(out, in_):
            nc.scalar.activation(out, in_, AF.Copy)

        def qk_chain(p, which, on_act=False):
            """One projection chain: which = 2*is_k + half."""
            is_k, half = which // 2, which % 2
            cols = bass.ts(half, 512)
            w, src_t, dst = ((wk_sb, kt, kTh) if is_k else (wq_sb, xt, qTh))
            ps = ps_s.tile([128, 512], F32, tag="s", name=f"qk{p}_{which}")
            for ct in range(CT):
                nc.tensor.matmul(
                    ps, w[:, ct, bass.ts(p, 128)], src_t[:, ct, cols],
                    start=(ct == 0), stop=(ct == CT - 1))
            (act_copy if on_act else nc.vector.tensor_copy)(dst[:, p, cols], ps)

        def qk_proj(p, on_act):
            for which in range(4):
                qk_chain(p, which, on_act)

        def v_proj_all():
            for mt in range(NT):
                v_chain(mt)

        def v_chain(mt):
            ps = ps_s.tile([128, HPC, HD], F32, tag="s", name=f"v{mt}")
            for ct in range(CT):
                nc.tensor.matmul(
                    ps, kt[:, ct, bass.ts(mt, 128)], wv_sb[:, ct, :],
                    start=(ct == 0), stop=(ct == CT - 1))
            nc.vector.tensor_copy(v_sb[:, mt, :, 0:HD], ps)

        t_tiles = {}
        e2_tiles = {}
        oth_tiles = {}
        fillers = {}

        e1_tiles = {}
        r1_tiles = {}

        def ad_block(P, mt):
            """One mt block: transposed scores -> exp1 -> t = e1 * A^T on
            DVE/Pool, plus the r1 accumulation matmuls (all-ones weight,
            broadcast over partitions) for the PREVIOUS mt block, so the
            first block of a pair can be pre-emitted into the prior pair's
            tail without holding extra r1 psum slots."""
            e1p = e1_pool.tile([128, 2, N], BF, tag="e1", name=f"e1_{P}_{mt}")
            e1_tiles[(P, mt)] = e1p
            tp = t_pool.tile([128, 2, N], BF, tag="t", name=f"t_{P}_{mt}")
            t_tiles[(P, mt)] = tp
            for hh in range(2):
                off = hh * 64
                s = ps_s.tile([128, N], F32, tag="s", name=f"s{P}_{mt}_{hh}")
                for mc in range(2):
                    nc.tensor.matmul(
                        s[:, bass.ts(mc, 512)],
                        kTh[off:off + 64, P, bass.ts(mt, 128)],
                        qTh[off:off + 64, P, bass.ts(mc, 512)],
                        start=True, stop=True)
                nc.scalar.activation(e1p[:, hh, :], s, AF.Exp, scale=SCALE)
            if mt > 0:
                ad_r1(P, mt - 1)
            for hh in range(2):
                eng = nc.vector if hh == 0 else nc.gpsimd
                eng.tensor_mul(tp[:, hh, :], e1p[:, hh, :], a_sb[:, mt, :])
            for fill in fillers.pop((P, mt), []):
                fill()

        def ad_r1(P, mt):
            e1p = e1_tiles.pop((P, mt))
            for hh in range(2):
                for half in range(2):
                    if mt == 0:
                        r1_tiles[(P, hh, half)] = ps_sm.tile(
                            [128, 512], F32, tag="sm", name=f"r1_{P}_{hh}_{half}")
                    nc.tensor.matmul(
                        r1_tiles[(P, hh, half)], ones,
                        e1p[:, hh, bass.ts(half, 512)],
                        start=(mt == 0), stop=(mt == NT - 1),
                        skip_group_check=True)

        def ad_tail(P):
            """Close the r1 chain, take reciprocals, apply *rc1 in place.
            The previous pair's output stage is interleaved per-mt so its
            DVE evacuations don't queue behind the whole TT2 stretch."""
            ad_r1(P, NT - 1)
            rc1 = rc1_pool.tile([128, 2, N], BF, tag="rc1", name=f"rc1_{P}")
            for hh in range(2):
                for half in range(2):
                    with nc.allow_low_precision(reason="softmax1 normalizer bf16"):
                        nc.vector.reciprocal(
                            rc1[:, hh, bass.ts(half, 512)],
                            r1_tiles.pop((P, hh, half)))
            if P >= 1:
                ef_begin(P - 1)
            for mt in range(NT):
                tp = t_tiles[(P, mt)]
                nc.vector.tensor_mul(tp[:, :, :], tp[:, :, :], rc1)
                if P >= 1:
                    ef_nt(P - 1, mt)
            if P >= 1:
                ef_end(P - 1)

        def ad_exp2(P):
            """exp2, in place on the merged head-pair t tiles [128, 2048]."""
            for mt in range(NT):
                tp = t_tiles.pop((P, mt))
                e2_tiles[(P, mt)] = tp
                nc.scalar.activation(tp[:, :, :], tp[:, :, :], AF.Exp)

        ef_state = {}

        def ef_begin(P):
            oth = oth_pool.tile([128, N], BF, tag="oth", name=f"oth{P}")
            oth_tiles[P] = oth
            ef_state[P] = (oth, [])

        def ef_tp(P, nt):
            oth, osbs = ef_state[P]
            pt = ps_sm.tile([128, 128], BF, tag="sm", name=f"pt{P}_{nt}")
            nc.tensor.transpose(pt, osbs[nt][:, :, :], ident)
            nc.vector.tensor_copy(oth[:, bass.ts(nt, 128)], pt)

        def ef_nt(P, nt):
            """attn@[v|1] for one n-tile (both heads): natural layout, col 64
            = r2, normalized on evacuation, transposed with lag 2."""
            oth, osbs = ef_state[P]
            osb = osb_pool.tile([128, 2, HD], BF, tag="osb", name=f"osb{P}_{nt}")
            osbs.append(osb)
            for hh in range(2):
                h = 2 * P + hh
                onat = ps_sm.tile([128, HD + 1], F32, tag="sm", name=f"o{h}_{nt}")
                for mt in range(NT):
                    nc.tensor.matmul(
                        onat,
                        e2_tiles[(P, mt)][:, hh, bass.ts(nt, 128)],
                        v_sb[:, mt, h, :],
                        start=(mt == 0), stop=(mt == NT - 1))
                rc2 = rc2_pool.tile([128, 1], F32, tag="rc2", name=f"rc2_{h}_{nt}")
                nc.vector.reciprocal(rc2, onat[:, HD:HD + 1])
                nc.vector.tensor_scalar_mul(osb[:, hh, :], onat[:, 0:HD], rc2)
            if nt >= 2:
                ef_tp(P, nt - 2)

        def ef_end(P):
            ef_tp(P, NT - 2)
            ef_tp(P, NT - 1)
            for mt in range(NT):
                e2_tiles.pop((P, mt))

        # Pipeline: each pair's first block is pre-emitted into the previous
        # pair's tail so ACT's exp1 stream never stalls at the rc1/TT2
        # handoff. qk/v projection chains are spread as per-block PE fillers
        # so no single PE wall starves ACT; the exp2 tail window absorbs the
        # previous pair's output stage.
        fillers.update({
            (0, 2): [lambda: qk_chain(1, 0), lambda: qk_chain(1, 1)],
            (0, 3): [lambda: qk_chain(1, 2), lambda: qk_chain(1, 3)],
            (1, 1): [lambda: v_chain(0)], (1, 3): [lambda: v_chain(1)],
            (1, 5): [lambda: v_chain(2)], (1, 6): [lambda: v_chain(3)],
        })
        qk_proj(0, on_act=True)
        for mt in range(NT):
            ad_block(0, mt)
        ad_block(1, 0)
        ad_tail(0)
        for mt in range(4, NT):
            v_chain(mt)
        ad_exp2(0)
        for P in range(1, NP):
            if P + 1 < NP:
                fillers.update({
                    (P, 2): [lambda w=w, p=P + 1: qk_chain(p, w) for w in (0, 1)],
                    (P, 4): [lambda w=w, p=P + 1: qk_chain(p, w) for w in (2, 3)],
                })
            for mt in range(1, NT):
                ad_block(P, mt)
            if P + 1 < NP:
                ad_block(P + 1, 0)
            ad_tail(P)
            if P + 1 < NP:
                ad_exp2(P)

        # Pair-3 drain, split by head halves: av/normalize for half 0 runs
        # under half 1's exp2 stream instead of after it.
        LP = NP - 1
        oth = oth_pool.tile([128, N], BF, tag="oth", name=f"oth{LP}")
        oth_tiles[LP] = oth
        osbs = [osb_pool.tile([128, 2, HD], BF, tag="osb", name=f"osbL{nt}")
                for nt in range(NT)]

        def drain_half(hh, tp_lag=None):
            h = 2 * LP + hh
            for mt in range(NT):
                nc.scalar.activation(
                    t_tiles[(LP, mt)][:, hh, :], t_tiles[(LP, mt)][:, hh, :],
                    AF.Exp)
            for nt in range(NT):
                onat = ps_sm.tile([128, HD + 1], F32, tag="sm", name=f"oL{hh}_{nt}")
                for mt in range(NT):
                    nc.tensor.matmul(
                        onat,
                        t_tiles[(LP, mt)][:, hh, bass.ts(nt, 128)],
                        v_sb[:, mt, h, :],
                        start=(mt == 0), stop=(mt == NT - 1))
                rc2 = rc2_pool.tile([128, 1], F32, tag="rc2", name=f"rc2L{hh}_{nt}")
                nc.vector.reciprocal(rc2, onat[:, HD:HD + 1])
                nc.vector.tensor_scalar_mul(osbs[nt][:, hh, :], onat[:, 0:HD], rc2)
                if tp_lag is not None and nt >= 2:
                    tp_lag(nt - 2)

        def drain_tp(nt):
            pt = ps_sm.tile([128, 128], BF, tag="sm", name=f"ptL{nt}")
            nc.tensor.transpose(pt, osbs[nt][:, :, :], ident)
            nc.vector.tensor_copy(oth[:, bass.ts(nt, 128)], pt)

        drain_half(0)
        drain_half(1, tp_lag=drain_tp)
        drain_tp(NT - 2)
        drain_tp(NT - 1)
        for mt in range(NT):
            t_tiles.pop((LP, mt))

        # ---- final projection: outT[c, n] = sum_P wp[P]^T @ oTh[P] ----
        for co in range(CT):
            ps = ps_s.tile([128, N], F32, tag="s", name=f"fin{co}")
            for half in range(2):
                cols = bass.ts(half, 512)
                for P in range(NP):
                    nc.tensor.matmul(
                        ps[:, cols], wp_sb[:, P, bass.ts(co, 128)],
                        oth_tiles[P][:, cols],
                        start=(P == 0), stop=(P == NP - 1),
                        skip_group_check=True)
            f = fin_pool.tile([128, N], BF, tag="f")
            if co % 2 == 0:
                nc.vector.tensor_copy(f, ps)
                nc.sync.dma_start(out=outT[co * 128:(co + 1) * 128, :], in_=f)
            else:
                act_copy(f, ps)
                nc.scalar.dma_start(out=outT[co * 128:(co + 1) * 128, :], in_=f)


def _prep(inputs):
    """Host-side shard prep: slice/transpose/cast per core."""
    x = np.asarray(inputs["x"], np.float32)
    k_in = np.asarray(inputs["k_in"], np.float32)
    attn_add = np.asarray(inputs["attn_add"], np.float32)
    Wq = np.asarray(inputs["Wq"], np.float32)
    Wkv = np.asarray(inputs["Wkv"], np.float32)
    Wproj = np.asarray(inputs["Wproj"], np.float32)
    bf = ml_dtypes.bfloat16
    in_maps = []
    for core in range(8):
        b, g = core // 2, core % 2
        h0 = g * HPC * HD  # column offset of this core's heads
        in_maps.append({
            "xT": np.ascontiguousarray(x[b].T).astype(bf),
            "kT": np.ascontiguousarray(k_in[b].T).astype(bf),
            "AT": np.ascontiguousarray(attn_add[b].T).astype(bf),
            "wq": np.ascontiguousarray(Wq[:, h0:h0 + HPC * HD]).astype(bf),
            "wk": np.ascontiguousarray(Wkv[:, h0:h0 + HPC * HD]).astype(bf),
            "wv": np.ascontiguousarray(Wkv[:, C + h0:C + h0 + HPC * HD]).astype(bf),
            "wp": np.ascontiguousarray(Wproj[h0:h0 + HPC * HD, :]).astype(bf),
        })
    return in_maps


def kernel(**inputs):
    if "nc" not in _CACHE:
        _CACHE["nc"] = _build()
    nc = _CACHE["nc"]
    in_maps = _prep(inputs)
    res = run_bass_kernel_spmd(nc, in_maps, core_ids=list(range(8)))
    bproj = np.asarray(inputs["bproj"], np.float32)
    out = np.empty((B, N, C), np.float32)
    for b in range(B):
        acc = (res.results[2 * b]["outT"].astype(np.float32)
               + res.results[2 * b + 1]["outT"].astype(np.float32))
        out[b] = acc.T + bproj
    return out
